# revision 45
# baseline (speedup 1.0000x reference)
"""MultiHeadDecoder (moe_routing) Trainium2 kernel, v7.

Expert-parallel: each of 8 cores owns one head. Host groups samples by
head, pads to capacity C (multiple of 8), ships everything bf16 (PSUM
accumulates f32; tolerance 2e-2 vs bf16 wire error ~4e-3).

Both stages keep weights stationary in the PE and stream sample columns,
so PE time tracks real sample count:
  stage A:  ht[hc][hid,s]  = relu(sum_k W1[k,hc]^T @ X^T[k][:,s] + b1)
  stage B:  outT[of][of,s] = sum_hc W2[of,hc]^T @ ht[hc][:,s] + b2
Output is transposed ([out_feature, sample]); host untransposes.

Timing model (measured): rings start ~8.7/9.5/10.2us (sync/scalar/
gpsimd), per-ring rate ~140GB/s at 4KB lines, ~250GB/s at 8KB lines,
consumer sees a DMA completion ~0.9us after the last packet. The PE's
HAM duty ramp needs 3-4us of continuous busy and resets on >1us idle.

Schedule: xin ships as two k-major halves on sync+scalar so stage A's
k=0 matmuls gate only on the first half; stage A is emitted k-major
(8 concurrent PSUM accumulators) so the k=1 wave queues behind k=0
without stalling it. W2 of0-1/of2-3 halves follow on sync/scalar,
rest by need (b1 whole on gpsimd, b2 sync, b3 scalar). Warmup matmuls
bridge program start to xin arrival; trailing dummies (gated on the
last output tile) keep the clock up through the epilogue semaphore
storm. Outputs: of-pairs over 3 rings, final 4 tiles as singles.
Stage-A relu runs on DVE (add+max tensor_scalar), stage-B bias
alternates ACT/DVE.
"""

import numpy as np

import concourse.bass as bass
import concourse.mybir as mybir
from concourse import bacc
from concourse.tile import TileContext
from concourse.bass_utils import run_bass_kernel_spmd

IN_F, HID, OUT_F, N_HEADS, BATCH = 256, 512, 2048, 8, 4096
N_CORES = 8
P = 128
KI = IN_F // P      # 2 input-feature chunks
HC = HID // P       # 4 hidden chunks
OF = OUT_F // P     # 16 output-feature tiles

f32 = mybir.dt.float32
bf16 = mybir.dt.bfloat16

try:
    from ml_dtypes import bfloat16 as np_bf16
except ImportError:
    import jax.numpy as jnp
    np_bf16 = jnp.bfloat16

_NC_CACHE: dict = {}

WARM_PRE = 14    # 264-col warmups bridging to xin arrival
WARM_MID = 4     # bridge dummies between stage A and stage B
WARM_POST = 12   # trailing dummies keeping HAM hot through teardown


def build_nc(C: int):
    """Per-core Bass program for sample capacity C (multiple of 8)."""
    G = C // 2
    assert G <= 512
    KH = C + HID         # cols per k-half of xin: X^T k-chunk | W1 k-chunk
    NB = HC + OF         # bias cols
    NIN = KI * KH + NB
    W2C = HC * P         # 512 w2 cols per of-tile

    nc = bacc.Bacc("TRN2", target_bir_lowering=False, debug=False,
                   num_devices=N_CORES)
    xin = nc.dram_tensor("xin", [P, NIN], bf16, kind="ExternalInput")
    w2 = nc.dram_tensor("w2", [P, OF * W2C], bf16, kind="ExternalInput")
    outT = nc.dram_tensor("outT", [OF // 2, P, 2 * C], bf16,
                          kind="ExternalOutput")

    ident = mybir.ActivationFunctionType.Identity
    op_add = mybir.AluOpType.add
    op_max = mybir.AluOpType.max

    with TileContext(nc) as tc:
        with (
            tc.tile_pool(name="const", bufs=1) as const,
            tc.tile_pool(name="psumA", bufs=3, space="PSUM") as psumA,
            tc.tile_pool(name="psumB", bufs=4, space="PSUM") as psumB,
            tc.tile_pool(name="psumW", bufs=1, space="PSUM") as psumW,
            tc.tile_pool(name="outp", bufs=6) as outp,
        ):
            # Warmup matmuls on an uninitialized tile (values irrelevant).
            wsrc = const.tile([P, 264], bf16, tag="warm")
            nc.gpsimd.memset(wsrc[:, :1], 0.0)
            wps = psumW.tile([P, 264], f32, tag="warmps")
            for i in range(WARM_PRE):
                nc.tensor.matmul(wps[:], lhsT=wsrc[:, :P],
                                 rhs=wsrc[:], start=True, stop=True)

            # --- input DMAs ---
            # ring starts stagger ~8.7/9.5/10.2us (sync/scalar/gpsimd);
            # schedule transfers so each w2 block beats its stage-B need.
            xs = const.tile([P, NIN], bf16, tag="xin")
            w2s = const.tile([P, OF * W2C], bf16, tag="w2s")
            HB = 2 * W2C

            def w2_dma(eng, c0, c1):
                eng.dma_start(w2s[:, c0:c1], w2[:, c0:c1])

            nc.sync.dma_start(xs[:, :KH], xin[:, :KH])
            nc.scalar.dma_start(xs[:, KH:], xin[:, KH:])
            w2_dma(nc.gpsimd, 0, HB)           # of0-1
            w2_dma(nc.sync, HB, 2 * HB)        # of2-3
            w2_dma(nc.sync, 2 * HB, 4 * HB)    # of4-7
            w2_dma(nc.scalar, 4 * HB, 6 * HB)  # of8-11
            w2_dma(nc.gpsimd, 6 * HB, 8 * HB)  # of12-15 (sync/scalar then
            # carry the output pairs without input backlog ahead of them)

            def xt_cols(k, g):
                base = k * KH + g * G
                return xs[:, base:base + G]

            def w1_tile(k, hc):
                base = k * KH + C + hc * P
                return xs[:, base:base + P]

            def w2_tile(of, hc):
                b = of * W2C + hc * P
                return w2s[:, b:b + P]
            # (w2s columns are of-major: of*512 + hc*128 + oc)

            # biases ship as bf16 at the tail of xin; convert once to f32
            bconv = const.tile([P, NB], f32, tag="bconv")
            nc.gpsimd.tensor_scalar_add(bconv[:], xs[:, KI * KH:NIN], 0.0)
            b1_s = bconv[:, 0:HC]
            b2_s = bconv[:, HC:NB]

            # --- stage A: ht[hc] = relu(X @ W1 + b1)^T on DVE ---
            # k-major within hc-pair waves: the k=0 matmuls gate only on
            # xin's first half while k=1 queues behind (4 live psum banks).
            hts = [const.tile([P, C], bf16, tag=f"ht{hc}", name=f"ht{hc}")
                   for hc in range(HC)]
            for wave in ((0,), (1,), (2,), (3,)):
                pssA = {(hc, g): psumA.tile([P, G], f32, tag="psA",
                                            name=f"psA{hc}_{g}")
                        for hc in wave for g in range(2)}
                for k in range(KI):
                    for hc in wave:
                        for g in range(2):
                            nc.tensor.matmul(
                                pssA[hc, g][:],
                                lhsT=w1_tile(k, hc),
                                rhs=xt_cols(k, g),
                                start=(k == 0), stop=(k == KI - 1),
                            )
                for hc in wave:
                    for g in range(2):
                        nc.vector.tensor_scalar(
                            hts[hc][:, g * G:(g + 1) * G], pssA[hc, g][:],
                            b1_s[:, hc:hc + 1], 0.0, op_add, op_max,
                        )

            # Bridge dummies: cover the w2(of0-1) completion gap.
            for i in range(WARM_MID):
                nc.tensor.matmul(wps[:], lhsT=wsrc[:, :P],
                                 rhs=wsrc[:], start=True, stop=True)

            # --- stage B: outT[of] = (H @ W2 + b2)^T, bf16 ---
            # pairs alternate sync/scalar; final singles rotate all rings
            pair_rings = [nc.sync, nc.scalar]
            tail_rings = [nc.gpsimd, nc.scalar, nc.gpsimd, nc.sync]
            ot = None
            for of in range(OF):
                pss = [psumB.tile([P, G], f32, tag="psB", name=f"psB{of}_{g}")
                       for g in range(2)]
                for hc in range(HC):
                    for g in range(2):
                        nc.tensor.matmul(
                            pss[g][:],
                            lhsT=w2_tile(of, hc),
                            rhs=hts[hc][:, g * G:(g + 1) * G],
                            start=(hc == 0), stop=(hc == HC - 1),
                        )
                if of % 2 == 0:
                    ot = outp.tile([P, 2 * C], bf16, tag="ot")
                off = (of % 2) * C
                for g in range(2):
                    dst = ot[:, off + g * G:off + (g + 1) * G]
                    if of % 2 == 1:
                        nc.scalar.activation(dst, pss[g][:], ident,
                                             bias=b2_s[:, of:of + 1])
                    else:
                        nc.vector.tensor_scalar_add(dst, pss[g][:],
                                                    b2_s[:, of:of + 1])
                pair = of // 2
                if of < OF - 4 and of % 2 == 1:
                    pair_rings[pair % 2].dma_start(outT[pair], ot[:])
                elif of >= OF - 4:
                    # final four ship individually, rotating rings
                    half = of % 2
                    tail_rings[of - (OF - 4)].dma_start(
                        outT[pair][:, half * C:(half + 1) * C],
                        ot[:, half * C:(half + 1) * C])

            # Trailing dummies, gated on the last output tile so they run
            # during the output drain and keep HAM hot into the storm.
            for i in range(WARM_POST):
                nc.tensor.matmul(wps[:], lhsT=ot[:, :P],
                                 rhs=ot[:, :264], start=True, stop=True)

    nc.compile()
    return nc


def kernel(X, X_head_idx, W1, b1, W2, b2):
    X = np.ascontiguousarray(np.asarray(X, dtype=np.float32))
    idx = np.asarray(X_head_idx).astype(np.int64)
    W1 = np.asarray(W1, dtype=np.float32)
    b1 = np.asarray(b1, dtype=np.float32)
    W2 = np.asarray(W2, dtype=np.float32)
    b2 = np.asarray(b2, dtype=np.float32)

    batch = X.shape[0]
    counts = np.bincount(idx, minlength=N_HEADS)
    order = np.argsort(idx, kind="stable")
    positions = np.split(order, np.cumsum(counts)[:-1])

    C = max(64, int(-(-int(counts.max()) // 8)) * 8)
    if C not in _NC_CACHE:
        _NC_CACHE[C] = build_nc(C)
    nc = _NC_CACHE[C]

    KH = C + HID
    NB = HC + OF
    NIN = KI * KH + NB

    in_maps = []
    for h in range(N_HEADS):
        pos = positions[h]
        cnt = len(pos)
        xinf = np.zeros((P, NIN), dtype=np.float32)
        w1r = W1[h].reshape(KI, P, HID)
        if cnt:
            xk = X[pos].T.reshape(KI, P, cnt)  # [k, p, s]
        for k in range(KI):
            if cnt:
                xinf[:, k * KH:k * KH + cnt] = xk[k]
            xinf[:, k * KH + C:(k + 1) * KH] = w1r[k]
        xinf[:, KI * KH:KI * KH + HC] = b1[h].reshape(HC, P).T
        xinf[:, KI * KH + HC:] = b2[h].reshape(OF, P).T
        # w2 packed: [p, of*512 + hc*128 + oc] = W2[hc*128+p, of*128+oc]
        w2r = W2[h].reshape(HC, P, OF, P)              # [hc, p, of, oc]
        w2p = np.ascontiguousarray(np.transpose(w2r, (1, 2, 0, 3)))
        in_maps.append({
            "xin": xinf.astype(np_bf16),
            "w2": w2p.reshape(P, OF * HC * P).astype(np_bf16),
        })

    try:
        res = run_bass_kernel_spmd(nc, in_maps, list(range(N_CORES)))
    except Exception:
        res = run_bass_kernel_spmd(nc, in_maps, list(range(N_CORES)))

    out = np.empty((batch, OUT_F), dtype=np.float32)
    for h in range(N_HEADS):
        pos = positions[h]
        cnt = len(pos)
        if cnt:
            o = np.asarray(res.results[h]["outT"]).astype(np.float32)
            o = o.reshape(OF // 2, P, 2, C)            # [pair, p, half, s]
            o = np.transpose(o, (3, 0, 2, 1))          # [s, pair, half, p]
            o = o.reshape(C, OUT_F)
            out[pos] = o[:cnt]
    return out


# revision 47
# speedup vs baseline: 1.0362x; 1.0362x over previous
"""MultiHeadDecoder (moe_routing) Trainium2 kernel, v7.

Expert-parallel: each of 8 cores owns one head. Host groups samples by
head, pads to capacity C (multiple of 8), ships everything bf16 (PSUM
accumulates f32; tolerance 2e-2 vs bf16 wire error ~4e-3).

Both stages keep weights stationary in the PE and stream sample columns,
so PE time tracks real sample count:
  stage A:  ht[hc][hid,s]  = relu(sum_k W1[k,hc]^T @ X^T[k][:,s] + b1)
  stage B:  outT[of][of,s] = sum_hc W2[of,hc]^T @ ht[hc][:,s] + b2
Output is transposed ([out_feature, sample]); host untransposes.

Timing model (measured): rings start ~8.7/9.5/10.2us (sync/scalar/
gpsimd), per-ring rate ~140GB/s at 4KB lines, ~250GB/s at 8KB lines,
consumer sees a DMA completion ~0.9us after the last packet. The PE's
HAM duty ramp needs 3-4us of continuous busy and resets on >1us idle.

Schedule: xin ships as two k-major halves on sync+scalar so stage A's
k=0 matmuls gate only on the first half; stage A is emitted k-major
(8 concurrent PSUM accumulators) so the k=1 wave queues behind k=0
without stalling it. W2 of0-1/of2-3 halves follow on sync/scalar,
rest by need (b1 whole on gpsimd, b2 sync, b3 scalar). Warmup matmuls
bridge program start to xin arrival; trailing dummies (gated on the
last output tile) keep the clock up through the epilogue semaphore
storm. Outputs: of-pairs over 3 rings, final 4 tiles as singles.
Stage-A relu runs on DVE (add+max tensor_scalar), stage-B bias
alternates ACT/DVE.
"""

import numpy as np

import concourse.bass as bass
import concourse.mybir as mybir
from concourse import bacc
from concourse.tile import TileContext
from concourse.bass_utils import run_bass_kernel_spmd

IN_F, HID, OUT_F, N_HEADS, BATCH = 256, 512, 2048, 8, 4096
N_CORES = 8
P = 128
KI = IN_F // P      # 2 input-feature chunks
HC = HID // P       # 4 hidden chunks
OF = OUT_F // P     # 16 output-feature tiles

f32 = mybir.dt.float32
bf16 = mybir.dt.bfloat16

try:
    from ml_dtypes import bfloat16 as np_bf16
except ImportError:
    import jax.numpy as jnp
    np_bf16 = jnp.bfloat16

_NC_CACHE: dict = {}

WARM_PRE = 14    # 264-col warmups bridging to xin arrival
WARM_MID = 4     # bridge dummies between stage A and stage B
WARM_POST = 18   # trailing dummies keeping HAM hot through teardown


def build_nc(C: int):
    """Per-core Bass program for sample capacity C (multiple of 8)."""
    G = C // 2
    assert G <= 512
    KH = C + HID         # cols per k-half of xin: X^T k-chunk | W1 k-chunk
    NB = HC + OF         # bias cols
    NIN = KI * KH + NB
    W2C = HC * P         # 512 w2 cols per of-tile

    nc = bacc.Bacc("TRN2", target_bir_lowering=False, debug=False,
                   num_devices=N_CORES)
    xin = nc.dram_tensor("xin", [P, NIN], bf16, kind="ExternalInput")
    w2 = nc.dram_tensor("w2", [P, OF * W2C], bf16, kind="ExternalInput")
    outT = nc.dram_tensor("outT", [OF // 2, P, 2 * C], bf16,
                          kind="ExternalOutput")

    ident = mybir.ActivationFunctionType.Identity
    op_add = mybir.AluOpType.add
    op_max = mybir.AluOpType.max

    with TileContext(nc) as tc:
        with (
            tc.tile_pool(name="const", bufs=1) as const,
            tc.tile_pool(name="psumA", bufs=3, space="PSUM") as psumA,
            tc.tile_pool(name="psumB", bufs=4, space="PSUM") as psumB,
            tc.tile_pool(name="psumW", bufs=1, space="PSUM") as psumW,
            tc.tile_pool(name="outp", bufs=6) as outp,
        ):
            # Warmup matmuls on an uninitialized tile (values irrelevant).
            wsrc = const.tile([P, 264], bf16, tag="warm")
            nc.gpsimd.memset(wsrc[:, :1], 0.0)
            wps = psumW.tile([P, 264], f32, tag="warmps")
            for i in range(WARM_PRE):
                nc.tensor.matmul(wps[:], lhsT=wsrc[:, :P],
                                 rhs=wsrc[:], start=True, stop=True)

            # --- input DMAs ---
            # ring starts stagger ~8.7/9.5/10.2us (sync/scalar/gpsimd);
            # schedule transfers so each w2 block beats its stage-B need.
            xs = const.tile([P, NIN], bf16, tag="xin")
            w2s = const.tile([P, OF * W2C], bf16, tag="w2s")
            HB = 2 * W2C

            def w2_dma(eng, c0, c1):
                eng.dma_start(w2s[:, c0:c1], w2[:, c0:c1])

            nc.sync.dma_start(xs[:, :KH], xin[:, :KH])
            nc.scalar.dma_start(xs[:, KH:], xin[:, KH:])
            # Critical prefix (xin halves + of0-1) gets one transfer per
            # ring with nothing competing; the rest queues behind.
            w2_dma(nc.gpsimd, 0, HB)           # of0-1
            w2_dma(nc.sync, HB, 2 * HB)        # of2-3
            w2_dma(nc.scalar, 2 * HB, 4 * HB)  # of4-7
            w2_dma(nc.gpsimd, 4 * HB, 6 * HB)  # of8-11
            w2_dma(nc.sync, 6 * HB, 8 * HB)    # of12-15

            def xt_cols(k, g):
                base = k * KH + g * G
                return xs[:, base:base + G]

            def w1_tile(k, hc):
                base = k * KH + C + hc * P
                return xs[:, base:base + P]

            def w2_tile(of, hc):
                b = of * W2C + hc * P
                return w2s[:, b:b + P]
            # (w2s columns are of-major: of*512 + hc*128 + oc)

            # biases ship as bf16 at the tail of xin; convert once to f32
            bconv = const.tile([P, NB], f32, tag="bconv")
            nc.gpsimd.tensor_scalar_add(bconv[:], xs[:, KI * KH:NIN], 0.0)
            b1_s = bconv[:, 0:HC]
            b2_s = bconv[:, HC:NB]

            # --- stage A: ht[hc] = relu(X @ W1 + b1)^T on DVE ---
            # k-major within hc-pair waves: the k=0 matmuls gate only on
            # xin's first half while k=1 queues behind (4 live psum banks).
            hts = [const.tile([P, C], bf16, tag=f"ht{hc}", name=f"ht{hc}")
                   for hc in range(HC)]
            for wave in ((0,), (1,), (2,), (3,)):
                pssA = {(hc, g): psumA.tile([P, G], f32, tag="psA",
                                            name=f"psA{hc}_{g}")
                        for hc in wave for g in range(2)}
                for k in range(KI):
                    for hc in wave:
                        for g in range(2):
                            nc.tensor.matmul(
                                pssA[hc, g][:],
                                lhsT=w1_tile(k, hc),
                                rhs=xt_cols(k, g),
                                start=(k == 0), stop=(k == KI - 1),
                            )
                for hc in wave:
                    for g in range(2):
                        nc.vector.tensor_scalar(
                            hts[hc][:, g * G:(g + 1) * G], pssA[hc, g][:],
                            b1_s[:, hc:hc + 1], 0.0, op_add, op_max,
                        )

            # Bridge dummies: cover the w2(of0-1) completion gap.
            for i in range(WARM_MID):
                nc.tensor.matmul(wps[:], lhsT=wsrc[:, :P],
                                 rhs=wsrc[:], start=True, stop=True)

            # --- stage B: outT[of] = (H @ W2 + b2)^T, bf16 ---
            # pairs alternate sync/scalar; final singles rotate all rings
            pair_rings = [nc.sync, nc.scalar]
            tail_rings = [nc.gpsimd, nc.scalar, nc.gpsimd, nc.sync]
            ot = None
            for of in range(OF):
                pss = [psumB.tile([P, G], f32, tag="psB", name=f"psB{of}_{g}")
                       for g in range(2)]
                for hc in range(HC):
                    for g in range(2):
                        nc.tensor.matmul(
                            pss[g][:],
                            lhsT=w2_tile(of, hc),
                            rhs=hts[hc][:, g * G:(g + 1) * G],
                            start=(hc == 0), stop=(hc == HC - 1),
                        )
                if of % 2 == 0:
                    ot = outp.tile([P, 2 * C], bf16, tag="ot")
                off = (of % 2) * C
                for g in range(2):
                    dst = ot[:, off + g * G:off + (g + 1) * G]
                    if of % 2 == 1:
                        nc.scalar.activation(dst, pss[g][:], ident,
                                             bias=b2_s[:, of:of + 1])
                    else:
                        nc.vector.tensor_scalar_add(dst, pss[g][:],
                                                    b2_s[:, of:of + 1])
                pair = of // 2
                if of < OF - 4 and of % 2 == 1:
                    pair_rings[pair % 2].dma_start(outT[pair], ot[:])
                elif of >= OF - 4:
                    # final four ship individually, rotating rings
                    half = of % 2
                    tail_rings[of - (OF - 4)].dma_start(
                        outT[pair][:, half * C:(half + 1) * C],
                        ot[:, half * C:(half + 1) * C])

            # Trailing dummies, gated on the last output tile so they run
            # during the output drain and keep HAM hot into the storm.
            for i in range(WARM_POST):
                nc.tensor.matmul(wps[:], lhsT=ot[:, :P],
                                 rhs=ot[:, :264], start=True, stop=True)

    nc.compile()
    return nc


def kernel(X, X_head_idx, W1, b1, W2, b2):
    X = np.ascontiguousarray(np.asarray(X, dtype=np.float32))
    idx = np.asarray(X_head_idx).astype(np.int64)
    W1 = np.asarray(W1, dtype=np.float32)
    b1 = np.asarray(b1, dtype=np.float32)
    W2 = np.asarray(W2, dtype=np.float32)
    b2 = np.asarray(b2, dtype=np.float32)

    batch = X.shape[0]
    counts = np.bincount(idx, minlength=N_HEADS)
    order = np.argsort(idx, kind="stable")
    positions = np.split(order, np.cumsum(counts)[:-1])

    C = max(64, int(-(-int(counts.max()) // 8)) * 8)
    if C not in _NC_CACHE:
        _NC_CACHE[C] = build_nc(C)
    nc = _NC_CACHE[C]

    KH = C + HID
    NB = HC + OF
    NIN = KI * KH + NB

    in_maps = []
    for h in range(N_HEADS):
        pos = positions[h]
        cnt = len(pos)
        xinf = np.zeros((P, NIN), dtype=np.float32)
        w1r = W1[h].reshape(KI, P, HID)
        if cnt:
            xk = X[pos].T.reshape(KI, P, cnt)  # [k, p, s]
        for k in range(KI):
            if cnt:
                xinf[:, k * KH:k * KH + cnt] = xk[k]
            xinf[:, k * KH + C:(k + 1) * KH] = w1r[k]
        xinf[:, KI * KH:KI * KH + HC] = b1[h].reshape(HC, P).T
        xinf[:, KI * KH + HC:] = b2[h].reshape(OF, P).T
        # w2 packed: [p, of*512 + hc*128 + oc] = W2[hc*128+p, of*128+oc]
        w2r = W2[h].reshape(HC, P, OF, P)              # [hc, p, of, oc]
        w2p = np.ascontiguousarray(np.transpose(w2r, (1, 2, 0, 3)))
        in_maps.append({
            "xin": xinf.astype(np_bf16),
            "w2": w2p.reshape(P, OF * HC * P).astype(np_bf16),
        })

    try:
        res = run_bass_kernel_spmd(nc, in_maps, list(range(N_CORES)))
    except Exception:
        res = run_bass_kernel_spmd(nc, in_maps, list(range(N_CORES)))

    out = np.empty((batch, OUT_F), dtype=np.float32)
    for h in range(N_HEADS):
        pos = positions[h]
        cnt = len(pos)
        if cnt:
            o = np.asarray(res.results[h]["outT"]).astype(np.float32)
            o = o.reshape(OF // 2, P, 2, C)            # [pair, p, half, s]
            o = np.transpose(o, (3, 0, 2, 1))          # [s, pair, half, p]
            o = o.reshape(C, OUT_F)
            out[pos] = o[:cnt]
    return out
